# revision 1
# baseline (speedup 1.0000x reference)
"""KIVI attention wrapper — Trainium2 Bass kernel, 8-way head-sharded.

Sharding: 16 heads / 8 cores = 2 heads per core (tensor parallel).
Per core: QKV^T in feature-major layout via PE-transposed X; KIVI 2-bit
fake-quant of K on device; scores computed transposed ([kpos, q]) so softmax
sum lands on a matmul ones-column; AllGather of per-core attention output;
c_proj column-sharded (each core produces 128 output columns, token-major
gathered on host).
"""
import sys
sys.path.insert(0, '/opt/trn_rl_repo')
import numpy as np

P = 128
TOK = 4096          # B*S = 4*1024
E = 1024
NB = 8              # embed 128-blocks
CH = 512            # token chunk
NCH = 8             # token 512-chunks
TB = 32             # token 128-blocks
MAGIC = 8388608.0   # 2^23: x + MAGIC - MAGIC == rint(x) for 0 <= x < 2^22
USE_F32R = True     # tf32 matmuls: 4x PE throughput, ~1e-3 component error

_CACHE = {}


def _build(sim_single=False):
    import concourse.bacc as bacc
    import concourse.mybir as mybir
    import concourse.tile as tile

    f32 = mybir.dt.float32
    fmm = mybir.dt.float32r if USE_F32R else mybir.dt.float32
    X = mybir.AxisListType.X
    ADD = mybir.AluOpType.add
    MULT = mybir.AluOpType.mult
    MAX = mybir.AluOpType.max
    SUB = mybir.AluOpType.subtract
    EXP = mybir.ActivationFunctionType.Exp

    nc = bacc.Bacc("TRN2", target_bir_lowering=False, debug=False,
                   num_devices=(1 if sim_single else 8))

    x_ap = nc.dram_tensor("x", [TOK, E], f32, kind="ExternalInput").ap()
    wqkv_ap = nc.dram_tensor("wqkv", [E, 384], f32, kind="ExternalInput").ap()
    bqkv_ap = nc.dram_tensor("bqkv", [P, 3], f32, kind="ExternalInput").ap()
    m8t_ap = nc.dram_tensor("m8t", [P, 32], f32, kind="ExternalInput").ap()
    wp_ap = nc.dram_tensor("wp", [E, P], f32, kind="ExternalInput").ap()
    bp_ap = nc.dram_tensor("bp", [P, 1], f32, kind="ExternalInput").ap()
    ident_ap = nc.dram_tensor("ident", [P, P], f32, kind="ExternalInput").ap()
    ones1_ap = nc.dram_tensor("ones1", [1, 64], f32, kind="ExternalInput").ap()
    yt_ap = nc.dram_tensor("yt", [P, TOK], f32, kind="ExternalOutput").ap()

    with tile.TileContext(nc) as tc:
        with tc.tile_pool(name="const", bufs=1) as constp, \
             tc.tile_pool(name="big", bufs=1) as bigp, \
             tc.tile_pool(name="dram", bufs=1, space="DRAM") as dramp:

            identt = constp.tile([P, P], f32)
            nc.sync.dma_start(identt[:], ident_ap)
            ones1t = constp.tile([1, 64], f32)
            nc.sync.dma_start(ones1t[:], ones1_ap)
            if USE_F32R:
                ones1r = constp.tile([1, 64], fmm, name="ones1r", tag="ones1r")
                nc.vector.tensor_copy(ones1r[:], ones1t[:])
            else:
                ones1r = ones1t
            m8tt = constp.tile([P, 32], f32)
            nc.sync.dma_start(m8tt[:], m8t_ap)
            bqkvt = constp.tile([P, 3], f32)
            nc.sync.dma_start(bqkvt[:], bqkv_ap)
            bpt = constp.tile([P, 1], f32)
            nc.sync.dma_start(bpt[:], bp_ap)
            onescol = constp.tile([P, 1], f32)
            nc.any.memset(onescol[:], 1.0)
            wts = []
            for eb in range(NB):
                wt = constp.tile([P, 384], f32, name=f"wt{eb}", tag=f"wt{eb}")
                nc.sync.dma_start(wt[:], wqkv_ap[eb * P:(eb + 1) * P, :])
                if USE_F32R:
                    wtr = constp.tile([P, 384], fmm, name=f"wtr{eb}",
                                      tag=f"wtr{eb}")
                    nc.vector.tensor_copy(wtr[:], wt[:])
                    wt = wtr
                wts.append(wt)
            wps = []
            for eb in range(NB):
                wpt = constp.tile([P, P], f32, name=f"wp{eb}", tag=f"wp{eb}")
                nc.sync.dma_start(wpt[:], wp_ap[eb * P:(eb + 1) * P, :])
                if USE_F32R:
                    wpr = constp.tile([P, P], fmm, name=f"wpr{eb}",
                                      tag=f"wpr{eb}")
                    nc.vector.tensor_copy(wpr[:], wpt[:])
                    wpt = wpr
                wps.append(wpt)

            # persistent feature-major tensors [128 = 2 heads x 64, 4096 tok]
            qT = bigp.tile([P, TOK], fmm, tag="qT")
            kT = bigp.tile([P, TOK], f32, tag="kT")
            vT = bigp.tile([P, TOK], f32, tag="vT")
            kdT = bigp.tile([P, TOK], fmm, tag="kdT")
            oT = bigp.tile([P, TOK], f32, tag="oT")
            qkvT = [qT, kT, vT]

            # ---------------- Stage 1: X^T + QKV^T GEMM ----------------
            with tc.tile_pool(name="s1", bufs=2) as s1p, \
                 tc.tile_pool(name="s1ps", bufs=5, space="PSUM") as s1ps, \
                 tc.tile_pool(name="g1ps", bufs=3, space="PSUM") as g1ps:
                for ch in range(NCH):
                    xns = []
                    for tb in range(4):
                        xn = s1p.tile([P, E], f32, name=f"xn{tb}", tag=f"xn{tb}")
                        nc.sync.dma_start(
                            xn[:], x_ap[ch * CH + tb * P: ch * CH + (tb + 1) * P, :])
                        xns.append(xn)
                    xTs = []
                    for eb in range(NB):
                        xT = s1p.tile([P, CH], fmm, name=f"xT{eb}", tag=f"xT{eb}")
                        xTs.append(xT)
                    for eb in range(NB):
                        for tb in range(4):
                            pst = s1ps.tile([P, P], f32, tag="pst")
                            nc.tensor.transpose(
                                pst[:], xns[tb][:, eb * P:(eb + 1) * P], identt[:])
                            dst = xTs[eb][:, tb * P:(tb + 1) * P]
                            if (eb + tb) % 2 == 0:
                                nc.vector.tensor_copy(dst, pst[:])
                            else:
                                nc.scalar.copy(dst, pst[:])
                    for m in range(3):
                        gps = g1ps.tile([P, CH], f32, tag="gps")
                        for eb in range(NB):
                            nc.tensor.matmul(
                                gps[:], wts[eb][:, m * P:(m + 1) * P], xTs[eb][:],
                                start=(eb == 0), stop=(eb == NB - 1))
                        nc.vector.tensor_tensor(
                            qkvT[m][:, ch * CH:(ch + 1) * CH], gps[:],
                            bqkvt[:, m:m + 1].to_broadcast((P, CH)), ADD)

            # ---------------- Stage 2: KIVI fake-quant of K -------------
            # ---------------- Stage 3: V transpose (+ones col) ----------
            vt_tiles = []
            with tc.tile_pool(name="s2", bufs=2) as s2p, \
                 tc.tile_pool(name="s2ps", bufs=2, space="PSUM") as s2ps:
                for kb in range(TB):
                    ps_a = s2ps.tile([P, P], f32, tag="ps_a")
                    nc.tensor.transpose(ps_a[:], kT[:, kb * P:(kb + 1) * P], identt[:])
                    knat = s2p.tile([P, P], f32, tag="knat")
                    nc.scalar.copy(knat[:], ps_a[:])
                    gmax = s2p.tile([P, 32], f32, tag="gmax")
                    nc.vector.tensor_reduce(
                        gmax[:], knat[:].rearrange("p (g f) -> p g f", f=4),
                        axis=X, op=MAX, apply_absolute_value=True)
                    scalet = s2p.tile([P, 32], f32, tag="scalet")
                    nc.vector.tensor_scalar_mul(scalet[:], gmax[:], 1.0 / 1.5)
                    rs = s2p.tile([P, 32], f32, tag="rs")
                    nc.vector.reciprocal(rs[:], scalet[:])
                    kd = s2p.tile([P, P], f32, tag="kd")
                    kd_g = kd[:].rearrange("p (g f) -> p g f", f=4)
                    knat_g = knat[:].rearrange("p (g f) -> p g f", f=4)
                    nc.vector.tensor_tensor(
                        kd_g, knat_g, rs[:, :, None].to_broadcast((P, 32, 4)), MULT)
                    nc.vector.tensor_scalar(kd[:], kd[:], 1.5, MAGIC,
                                            ADD, ADD)
                    nc.vector.tensor_scalar(kd[:], kd[:], MAGIC, 1.5,
                                            SUB, SUB)
                    nc.vector.tensor_tensor(
                        kd_g, kd_g, scalet[:, :, None].to_broadcast((P, 32, 4)), MULT)
                    ps_b = s2ps.tile([P, P], f32, tag="ps_b")
                    nc.tensor.transpose(ps_b[:], kd[:], identt[:])
                    nc.scalar.copy(kdT[:, kb * P:(kb + 1) * P], ps_b[:])

                    # V natural tiles, one per head, with ones column at 64
                    ps_v = s2ps.tile([P, P], f32, tag="ps_v")
                    nc.tensor.transpose(ps_v[:], vT[:, kb * P:(kb + 1) * P], identt[:])
                    vh = []
                    for h in range(2):
                        v = bigp.tile([P, 65], fmm, name=f"v{kb}_{h}",
                                      tag=f"v{kb}_{h}")
                        nc.vector.tensor_copy(v[:, 64:65], onescol[:])
                        nc.scalar.copy(
                            v[:, 0:64], ps_v[:, h * 64:(h + 1) * 64])
                        vh.append(v)
                    vt_tiles.append(vh)

            # ---------------- Stage 4: attention ------------------------
            with tc.tile_pool(name="s4", bufs=2) as s4p, \
                 tc.tile_pool(name="s4ps", bufs=3, space="PSUM") as s4ps, \
                 tc.tile_pool(name="avps", bufs=2, space="PSUM") as avps, \
                 tc.tile_pool(name="rps", bufs=2, space="PSUM") as rps:
                for b in range(4):
                    for h in range(2):
                        hs = slice(h * 64, (h + 1) * 64)
                        for qc in range(2):
                            q0 = b * 1024 + qc * CH
                            es = []
                            for kb in range(8):
                                gkb = b * 8 + kb
                                ps_s = s4ps.tile([P, CH], f32, tag="ps_s")
                                nc.tensor.matmul(
                                    ps_s[:],
                                    kdT[hs, gkb * P:(gkb + 1) * P],
                                    qT[hs, q0:q0 + CH],
                                    start=True, stop=True)
                                e = s4p.tile([P, CH], fmm, name=f"e{kb}",
                                             tag=f"e{kb}")
                                nc.scalar.activation(
                                    e[:], ps_s[:], EXP,
                                    bias=m8tt[:, gkb:gkb + 1], scale=0.125)
                                es.append(e)
                            ps_av = avps.tile([65, CH], f32, tag="ps_av")
                            for kb in range(8):
                                nc.tensor.matmul(
                                    ps_av[:], vt_tiles[b * 8 + kb][h][:], es[kb][:],
                                    start=(kb == 0), stop=(kb == 7))
                            rS = s4p.tile([1, CH], fmm, tag="rS")
                            with nc.allow_low_precision(reason="tf32 recip"):
                                nc.vector.reciprocal(rS[:], ps_av[64:65, :])
                            ps_r = rps.tile([64, CH], f32, tag="ps_r")
                            nc.tensor.matmul(ps_r[:], ones1r[:], rS[:],
                                             start=True, stop=True)
                            rrep = s4p.tile([64, CH], f32, tag="rrep")
                            nc.scalar.copy(rrep[:], ps_r[:])
                            nc.vector.tensor_tensor(
                                oT[hs, q0:q0 + CH], ps_av[0:64, :], rrep[:], MULT)

            # ---------------- Stage 5: AllGather + c_proj ----------------
            agin = dramp.tile([P, TOK], f32, tag="agin")
            agout = dramp.tile([8, P, TOK], f32, tag="agout",
                               addr_space=("Local" if sim_single else "Shared"))
            nc.gpsimd.dma_start(agin[:], oT[:])
            if sim_single:
                for r in range(8):
                    nc.gpsimd.dma_start(agout[r], agin[:])
            else:
                nc.gpsimd.collective_compute(
                    "AllGather", mybir.AluOpType.bypass,
                    replica_groups=[list(range(8))],
                    ins=[agin[:]], outs=[agout[:]])
            with tc.tile_pool(name="s5", bufs=3) as s5p, \
                 tc.tile_pool(name="s5ps", bufs=2, space="PSUM") as s5ps:
                for nch in range(NCH):
                    ps_p = s5ps.tile([P, CH], f32, tag="ps_p")
                    for kb2 in range(NB):
                        rt = s5p.tile([P, CH], f32, tag="rt")
                        nc.gpsimd.dma_start(
                            rt[:], agout[kb2, :, nch * CH:(nch + 1) * CH])
                        if USE_F32R:
                            rtr = s5p.tile([P, CH], fmm, tag="rtr")
                            if kb2 % 2 == 0:
                                nc.vector.tensor_copy(rtr[:], rt[:])
                            else:
                                nc.scalar.copy(rtr[:], rt[:])
                            rt = rtr
                        nc.tensor.matmul(ps_p[:], wps[kb2][:], rt[:],
                                         start=(kb2 == 0), stop=(kb2 == NB - 1))
                    yts = s5p.tile([P, CH], f32, tag="yts")
                    nc.vector.tensor_tensor(
                        yts[:], ps_p[:], bpt[:].to_broadcast((P, CH)), ADD)
                    nc.sync.dma_start(yt_ap[:, nch * CH:(nch + 1) * CH], yts[:])

    nc.compile()
    return nc


def make_in_maps(hidden_states, attention_mask, w_attn, b_attn, w_proj, b_proj):
    x = np.ascontiguousarray(np.asarray(hidden_states, np.float32).reshape(TOK, E))
    mask = np.asarray(attention_mask, np.float32)
    wa = np.asarray(w_attn, np.float32)
    ba = np.asarray(b_attn, np.float32)
    wpf = np.asarray(w_proj, np.float32)
    bp = np.asarray(b_proj, np.float32)

    m8 = (mask * np.float32(0.125)).reshape(4, 8, 128)
    m8t = np.ascontiguousarray(m8.transpose(2, 0, 1).reshape(128, 32))
    ident = np.eye(P, dtype=np.float32)
    ones1 = np.ones((1, 64), dtype=np.float32)

    in_maps = []
    for c in range(8):
        cs = slice(c * P, (c + 1) * P)
        wqkv = np.ascontiguousarray(np.concatenate(
            [wa[:, cs], wa[:, 1024 + c * P:1024 + (c + 1) * P],
             wa[:, 2048 + c * P:2048 + (c + 1) * P]], axis=1))
        bqkv = np.ascontiguousarray(np.stack(
            [ba[cs], ba[1024 + c * P:1024 + (c + 1) * P],
             ba[2048 + c * P:2048 + (c + 1) * P]], axis=1))
        in_maps.append({
            "x": x, "wqkv": wqkv, "bqkv": bqkv, "m8t": m8t,
            "wp": np.ascontiguousarray(wpf[:, cs]),
            "bp": np.ascontiguousarray(bp[cs][:, None]),
            "ident": ident, "ones1": ones1,
        })
    return in_maps


def kernel(hidden_states, attention_mask, w_attn, b_attn, w_proj, b_proj):
    from concourse import bass_utils
    if "nc" not in _CACHE:
        _CACHE["nc"] = _build()
    nc = _CACHE["nc"]
    in_maps = make_in_maps(hidden_states, attention_mask, w_attn, b_attn,
                           w_proj, b_proj)
    res = bass_utils.run_bass_kernel_spmd(nc, in_maps, core_ids=list(range(8)))
    y = np.empty((TOK, E), dtype=np.float32)
    for c in range(8):
        y[:, c * P:(c + 1) * P] = res.results[c]["yt"].T
    return y.reshape(4, 1024, E)



# revision 24
# speedup vs baseline: 32.5748x; 32.5748x over previous
"""KIVI attention wrapper — Trainium2 Bass kernel, 8-way head-sharded (v2).

Sharding: 16 heads / 8 cores = 2 heads per core (tensor parallel).
Host passes X pre-transposed (xT [E, TOK]) so the kernel never transposes X on
device.  Per chunk of 512 tokens the QKV^T GEMM, KIVI 2-bit fake-quant of K
(read straight out of PSUM), and V re-transpose are fused into one pipelined
stage.  Attention runs per batch with per-head score matmuls issued to
disjoint PE row groups; softmax sum comes from a ones-column folded into V.
The attention output is AllGathered per batch (4 small collectives that
overlap with compute of later batches) and c_proj is column-sharded.
All DRAM tensors that feed matmuls are declared float32r so no on-device
dtype-conversion copies are needed anywhere.
"""
import sys
sys.path.insert(0, '/opt/trn_rl_repo')
import numpy as np
import ml_dtypes

P = 128
TOK = 4096          # B*S = 4*1024
E = 1024
NB = 8              # embed 128-blocks
CH = 512            # token chunk
NCH = 8             # token 512-chunks
SB = 1024           # tokens per batch sample
MAGIC = 8388608.0   # 2^23: x + MAGIC - MAGIC == rint(x) for 0 <= x < 2^22

_CACHE = {}


def _build(sim_single=False):
    import concourse.bacc as bacc
    import concourse.mybir as mybir
    import concourse.tile as tile

    f32 = mybir.dt.float32
    fmm = mybir.dt.float32r
    bf16 = mybir.dt.bfloat16
    X = mybir.AxisListType.X
    ADD = mybir.AluOpType.add
    MULT = mybir.AluOpType.mult
    MAX = mybir.AluOpType.max
    SUB = mybir.AluOpType.subtract
    EXP = mybir.ActivationFunctionType.Exp

    nc = bacc.Bacc("TRN2", target_bir_lowering=False, debug=False,
                   num_devices=(1 if sim_single else 8))

    xT_ap = nc.dram_tensor("xT", [E, TOK], fmm, kind="ExternalInput").ap()
    wqkv_ap = nc.dram_tensor("wqkv", [E, 384], fmm, kind="ExternalInput").ap()
    bqkv_ap = nc.dram_tensor("bqkv", [P, 3], f32, kind="ExternalInput").ap()
    m8t_ap = nc.dram_tensor("m8t", [P, 32], f32, kind="ExternalInput").ap()
    wp_ap = nc.dram_tensor("wp", [E, P], bf16, kind="ExternalInput").ap()
    bp_ap = nc.dram_tensor("bp", [P, 1], f32, kind="ExternalInput").ap()
    ident_ap = nc.dram_tensor("ident", [P, P], f32, kind="ExternalInput").ap()
    ones1_ap = nc.dram_tensor("ones1", [1, 64], fmm, kind="ExternalInput").ap()
    chain_ap = nc.dram_tensor("chain", [P, TOK], f32, kind="ExternalInput").ap()
    yt_ap = nc.dram_tensor("yt", [P, TOK], f32, kind="ExternalOutput").ap()

    with tile.TileContext(nc) as tc:
        with tc.tile_pool(name="const", bufs=1) as constp, \
             tc.tile_pool(name="big", bufs=1) as bigp, \
             tc.tile_pool(name="dram", bufs=1, space="DRAM") as dramp:

            identt = constp.tile([P, P], f32)
            nc.sync.dma_start(identt[:], ident_ap)
            chaint = constp.tile([P, 1], f32)
            nc.sync.dma_start(chaint[:], chain_ap[:, 0:1])
            ones1t = constp.tile([1, 64], fmm)
            nc.sync.dma_start(ones1t[:], ones1_ap)
            m8tt = constp.tile([P, 32], f32)
            nc.sync.dma_start(m8tt[:], m8t_ap)
            bqkvt = constp.tile([P, 3], f32)
            nc.sync.dma_start(bqkvt[:], bqkv_ap)
            bpt = constp.tile([P, 1], f32)
            nc.sync.dma_start(bpt[:], bp_ap)
            onescol = constp.tile([P, 1], f32)
            nc.any.memset(onescol[:], 1.0)
            wts = []
            for eb in range(NB):
                wt = constp.tile([P, 384], fmm, name=f"wt{eb}", tag=f"wt{eb}")
                nc.scalar.dma_start(wt[:], wqkv_ap[eb * P:(eb + 1) * P, :])
                wts.append(wt)
            wps = []
            for eb in range(NB):
                wpt = constp.tile([P, P], bf16, name=f"wp{eb}", tag=f"wp{eb}")
                nc.scalar.dma_start(wpt[:], wp_ap[eb * P:(eb + 1) * P, :])
                wps.append(wpt)

            # persistent feature-major tensors [128 = 2 heads x 64, 4096 tok]
            qT = bigp.tile([P, TOK], fmm, tag="qT")
            kdT = bigp.tile([P, TOK], fmm, tag="kdT")
            oT = bigp.tile([P, TOK], bf16, tag="oT")
            # V natural tiles (one per 128-token block per head, ones col @64)
            vt_tiles = [[bigp.tile([P, 65], fmm, name=f"v{kb}_{h}",
                                   tag=f"v{kb}_{h}") for h in range(2)]
                        for kb in range(32)]

            # per-batch collective buffers
            agins = [dramp.tile([P, SB], bf16, name=f"agin{b}",
                      tag=f"agin{b}") for b in range(3)]
            agouts = [dramp.tile([8, P, SB], bf16, name=f"agout{b}", tag=f"agout{b}",
                                 addr_space=("Local" if sim_single else "Shared"))
                      for b in range(3)]
            agin3 = [dramp.tile([P, CH], bf16, name=f"agin3{h}",
                     tag=f"agin3{h}") for h in range(2)]
            agout3 = [dramp.tile([8, P, CH], bf16, name=f"agout3{h}",
                      tag=f"agout3{h}",
                      addr_space=("Local" if sim_single else "Shared"))
                      for h in range(2)]

            with tc.tile_pool(name="s1", bufs=2) as s1p, \
                 tc.tile_pool(name="sq", bufs=2) as sqp, \
                 tc.tile_pool(name="s4", bufs=2) as s4p, \
                 tc.tile_pool(name="es", bufs=1) as esp, \
                 tc.tile_pool(name="s5", bufs=1) as s5p, \
                 tc.tile_pool(name="gps", bufs=1, space="PSUM") as gpsp, \
                 tc.tile_pool(name="pst", bufs=2, space="PSUM") as pstp, \
                 tc.tile_pool(name="ps_s", bufs=1, space="PSUM") as pssp, \
                 tc.tile_pool(name="avps", bufs=1, space="PSUM") as avps, \
                 tc.tile_pool(name="rps", bufs=1, space="PSUM") as rps, \
                 tc.tile_pool(name="s5ps", bufs=1, space="PSUM") as s5ps:

                def stage1_chunk(ch):
                    """QKV^T GEMM + K quant + V transpose for tokens
                    [ch*512, (ch+1)*512)."""
                    xts = []
                    for eb in range(NB):
                        xt = s1p.tile([P, CH], fmm, name=f"xt{eb}",
                                      tag=f"xt{eb}")
                        nc.sync.dma_start(
                            xt[:], xT_ap[eb * P:(eb + 1) * P,
                                         ch * CH:(ch + 1) * CH])
                        xts.append(xt)
                    # Q
                    gq = gpsp.tile([P, CH], f32, tag="gqkv")
                    for eb in range(NB):
                        nc.tensor.matmul(gq[:], wts[eb][:, 0:P], xts[eb][:],
                                         start=(eb == 0), stop=(eb == NB - 1))
                    nc.vector.tensor_tensor(
                        qT[:, ch * CH:(ch + 1) * CH], gq[:],
                        bqkvt[:, 0:1].to_broadcast((P, CH)), ADD)
                    # K
                    gk = gpsp.tile([P, CH], f32, tag="gqkv")
                    for eb in range(NB):
                        nc.tensor.matmul(gk[:], wts[eb][:, P:2 * P], xts[eb][:],
                                         start=(eb == 0), stop=(eb == NB - 1))
                    kTc = s1p.tile([P, CH], f32, tag="kTc")
                    nc.vector.tensor_tensor(
                        kTc[:], gk[:], bqkvt[:, 1:2].to_broadcast((P, CH)), ADD)
                    # V
                    gv = gpsp.tile([P, CH], f32, tag="gqkv")
                    for eb in range(NB):
                        nc.tensor.matmul(gv[:], wts[eb][:, 2 * P:3 * P],
                                         xts[eb][:],
                                         start=(eb == 0), stop=(eb == NB - 1))
                    vTc = s1p.tile([P, CH], f32, tag="vTc")
                    nc.vector.tensor_tensor(
                        vTc[:], gv[:], bqkvt[:, 2:3].to_broadcast((P, CH)), ADD)

                    for j in range(4):
                        gkb = ch * 4 + j
                        # --- KIVI fake-quant of K block (token-major) ---
                        ps_a = pstp.tile([P, P], f32, tag="pst")
                        nc.tensor.transpose(
                            ps_a[:], kTc[:, j * P:(j + 1) * P], identt[:])
                        gmax = s1p.tile([P, 32], f32, tag="gmax")
                        nc.vector.tensor_reduce(
                            gmax[:], ps_a[:].rearrange("p (g f) -> p g f", f=4),
                            axis=X, op=MAX, apply_absolute_value=True)
                        scalet = s1p.tile([P, 32], f32, tag="scalet")
                        nc.vector.tensor_scalar_mul(scalet[:], gmax[:],
                                                    1.0 / 1.5)
                        rs = s1p.tile([P, 32], f32, tag="rs")
                        nc.vector.reciprocal(rs[:], scalet[:])
                        kd = s1p.tile([P, P], f32, tag="kd")
                        kd_g = kd[:].rearrange("p (g f) -> p g f", f=4)
                        nc.vector.tensor_tensor(
                            kd_g, ps_a[:].rearrange("p (g f) -> p g f", f=4),
                            rs[:, :, None].to_broadcast((P, 32, 4)), MULT)
                        nc.vector.tensor_scalar(kd[:], kd[:], 1.5, MAGIC,
                                                ADD, ADD)
                        nc.vector.tensor_scalar(kd[:], kd[:], MAGIC, 1.5,
                                                SUB, SUB)
                        nc.vector.tensor_tensor(
                            kd_g, kd_g,
                            scalet[:, :, None].to_broadcast((P, 32, 4)), MULT)
                        ps_b = pstp.tile([P, P], f32, tag="pst")
                        nc.tensor.transpose(ps_b[:], kd[:], identt[:])
                        nc.vector.tensor_copy(
                            kdT[:, gkb * P:(gkb + 1) * P], ps_b[:])

                        # --- V natural tiles with ones column ---
                        ps_v = pstp.tile([P, P], f32, tag="pst")
                        nc.tensor.transpose(
                            ps_v[:], vTc[:, j * P:(j + 1) * P], identt[:])
                        for h in range(2):
                            v = vt_tiles[gkb][h]
                            nc.vector.tensor_copy(v[:, 64:65], onescol[:])
                            nc.vector.tensor_copy(
                                v[:, 0:64], ps_v[:, h * 64:(h + 1) * 64])

                def stage4_batch(b, qcs=(0, 1)):
                    """attention for batch b (2 heads local), then AllGather."""
                    for qc in qcs:
                        q0 = b * SB + qc * CH
                        es = {}
                        for kb in range(8):
                            gkb = b * 8 + kb
                            for h in range(2):
                                hs = slice(h * 64, (h + 1) * 64)
                                ps_s = pssp.tile([P, CH], f32, tag=f"ps_s{h}")
                                nc.tensor.matmul(
                                    ps_s[:],
                                    kdT[hs, gkb * P:(gkb + 1) * P],
                                    qT[hs, q0:q0 + CH],
                                    start=True, stop=True)
                                e = esp.tile([P, CH], fmm, name=f"e{kb}_{h}",
                                             tag=f"e{kb}_{h}")
                                nc.scalar.activation(
                                    e[:], ps_s[:], EXP,
                                    bias=m8tt[:, gkb:gkb + 1], scale=0.125)
                                es[(kb, h)] = e
                        for h in range(2):
                            ps_av = avps.tile([65, CH], f32, tag="ps_av")
                            for kb in range(8):
                                nc.tensor.matmul(
                                    ps_av[:], vt_tiles[b * 8 + kb][h][:],
                                    es[(kb, h)][:],
                                    start=(kb == 0), stop=(kb == 7))
                            rS = s4p.tile([1, CH], fmm, tag="rS")
                            with nc.allow_low_precision(reason="tf32 recip"):
                                nc.vector.reciprocal(rS[:], ps_av[64:65, :])
                            ps_r = rps.tile([64, CH], f32, tag="ps_r")
                            nc.tensor.matmul(ps_r[:], ones1t[:], rS[:],
                                             start=True, stop=True)
                            rrep = s4p.tile([64, CH], f32, tag="rrep")
                            nc.vector.tensor_copy(rrep[:], ps_r[:])
                            hs = slice(h * 64, (h + 1) * 64)
                            nc.vector.tensor_tensor(
                                oT[hs, q0:q0 + CH], ps_av[0:64, :], rrep[:],
                                MULT)
                    if b == 3:
                        # half-batch AllGathers so the first half's c_proj
                        # overlaps the second half's attention (tail batch)
                        qc = qcs[-1]
                        t0 = b * SB + qc * CH
                        nc.gpsimd.dma_start(agin3[qc][:], oT[:, t0:t0 + CH])
                        if sim_single:
                            nc.gpsimd.dma_start(agout3[qc][0], agin3[qc][:])
                        else:
                            nc.gpsimd.collective_compute(
                                "AllGather", mybir.AluOpType.bypass,
                                replica_groups=[list(range(8))],
                                ins=[agin3[qc][:]], outs=[agout3[qc][:]])
                        return
                    if qcs[-1] != 1:
                        return
                    # per-batch AllGather of this batch's attention output
                    nc.gpsimd.dma_start(agins[b][:], oT[:, b * SB:(b + 1) * SB])
                    if sim_single:
                        # local shard only - the real AllGather runs on TOPSP
                        # hardware, not on this core's engines
                        nc.gpsimd.dma_start(agouts[b][0], agins[b][:])
                    else:
                        nc.gpsimd.collective_compute(
                            "AllGather", mybir.AluOpType.bypass,
                            replica_groups=[list(range(8))],
                            ins=[agins[b][:]], outs=[agouts[b][:]])

                def stage5_batch(b):
                    """c_proj for batch b's tokens (128 output cols/core)."""
                    if b == 3:
                        for half in range(2):
                            t0 = b * SB + half * CH
                            rts = []
                            for kb2 in range(NB):
                                rt = s5p.tile([P, CH], bf16,
                                              name=f"rth{kb2}", tag=f"rth{kb2}")
                                nc.scalar.dma_start(rt[:], agout3[half][kb2])
                                rts.append(rt)
                            ps_p = s5ps.tile([P, CH], f32, tag="ps_p")
                            for kb2 in range(NB):
                                nc.tensor.matmul(
                                    ps_p[:], wps[kb2][:], rts[kb2][:],
                                    start=(kb2 == 0), stop=(kb2 == NB - 1))
                            yts = sqp.tile([P, CH], f32, tag="yts")
                            nc.vector.tensor_tensor(
                                yts[:], ps_p[:], bpt[:].to_broadcast((P, CH)),
                                ADD)
                            nc.gpsimd.dma_start(yt_ap[:, t0:t0 + CH], yts[:])
                        return
                    rts = []
                    for kb2 in range(NB):
                        rt = s5p.tile([P, SB], bf16, name=f"rt{kb2}",
                                      tag=f"rt{kb2}")
                        nc.scalar.dma_start(rt[:], agouts[b][kb2])
                        rts.append(rt)
                    for half in range(2):
                        t0 = b * SB + half * CH
                        ps_p = s5ps.tile([P, CH], f32, tag="ps_p")
                        for kb2 in range(NB):
                            nc.tensor.matmul(
                                ps_p[:], wps[kb2][:],
                                rts[kb2][:, half * CH:(half + 1) * CH],
                                start=(kb2 == 0), stop=(kb2 == NB - 1))
                        yts = sqp.tile([P, CH], f32, tag="yts")
                        nc.vector.tensor_tensor(
                            yts[:], ps_p[:], bpt[:].to_broadcast((P, CH)), ADD)
                        nc.gpsimd.dma_start(yt_ap[:, t0:t0 + CH], yts[:])

                # pipelined emission: QKV chunks for batch b, attention for
                # batch b, c_proj for batch b-1 (so its AllGather overlaps)
                for b in range(4):
                    stage1_chunk(2 * b)
                    stage1_chunk(2 * b + 1)
                    stage4_batch(b, qcs=(0,))
                    if b > 1:
                        # demote priority so the scheduler doesn't weave these
                        # rt-gated matmuls ahead of ready attention work
                        with tc.high_priority(offset=-1500):
                            stage5_batch(b - 2)
                    stage4_batch(b, qcs=(1,))
                with tc.high_priority(offset=-1500):
                    stage5_batch(2)
                    stage5_batch(3)

    nc.compile()
    return nc


def make_in_maps(hidden_states, attention_mask, w_attn, b_attn, w_proj, b_proj):
    x = np.asarray(hidden_states, np.float32).reshape(TOK, E)
    xT = np.ascontiguousarray(x.T)
    mask = np.asarray(attention_mask, np.float32)
    wa = np.asarray(w_attn, np.float32)
    ba = np.asarray(b_attn, np.float32)
    wpf = np.asarray(w_proj, np.float32)
    bp = np.asarray(b_proj, np.float32)

    m8 = (mask * np.float32(0.125)).reshape(4, 8, 128)
    m8t = np.ascontiguousarray(m8.transpose(2, 0, 1).reshape(128, 32))
    ident = np.eye(P, dtype=np.float32)
    ones1 = np.ones((1, 64), dtype=np.float32)

    in_maps = []
    for c in range(8):
        cs = slice(c * P, (c + 1) * P)
        wqkv = np.ascontiguousarray(np.concatenate(
            [wa[:, cs], wa[:, 1024 + c * P:1024 + (c + 1) * P],
             wa[:, 2048 + c * P:2048 + (c + 1) * P]], axis=1))
        bqkv = np.ascontiguousarray(np.stack(
            [ba[cs], ba[1024 + c * P:1024 + (c + 1) * P],
             ba[2048 + c * P:2048 + (c + 1) * P]], axis=1))
        in_maps.append({
            "xT": xT, "wqkv": wqkv, "bqkv": bqkv, "m8t": m8t,
            "wp": np.ascontiguousarray(wpf[:, cs]).astype(ml_dtypes.bfloat16),
            "bp": np.ascontiguousarray(bp[cs][:, None]),
            "ident": ident, "ones1": ones1,
            "chain": np.zeros((P, TOK), np.float32),
        })
    return in_maps


def kernel(hidden_states, attention_mask, w_attn, b_attn, w_proj, b_proj):
    from concourse import bass_utils
    if "nc" not in _CACHE:
        _CACHE["nc"] = _build()
    nc = _CACHE["nc"]
    in_maps = make_in_maps(hidden_states, attention_mask, w_attn, b_attn,
                           w_proj, b_proj)
    res = bass_utils.run_bass_kernel_spmd(nc, in_maps, core_ids=list(range(8)))
    y = np.empty((TOK, E), dtype=np.float32)
    for c in range(8):
        y[:, c * P:(c + 1) * P] = res.results[c]["yt"].T
    return y.reshape(4, 1024, E)
